# revision 27
# baseline (speedup 1.0000x reference)
"""Multi-head attention kernel for Trainium2, 8 NeuronCores.

Problem: B=4, T=2048, D_in=1024, 16 heads x 64 dim, E=1024 (fp32).

Sharding: (batch x head-group). Core c handles batch b=c//2 and head-group
g=c%2 (8 heads, 512 qk/v dims). Each core computes its batch's QKV
projections restricted to its heads, full attention for those heads, and a
partial output projection. The host sums the two partial projections per
batch (the only cross-core reduction) and stacks batches.

Per-core dataflow (all matmuls bf16 inputs, fp32 PSUM accumulation):
  xT      = dma-xbar-transpose(cast_bf16(x))            [1024, 2048] per tensor
  qhT/khT = w.T @ xT   (weights stationary)             [512, 2048]
  vh      = xT.T @ wv  (x stationary) + ones column     [2048, 8*65]
  S^T     = khT_h.T @ qhT_h per head pair               PSUM [128,1024]
  expS    = ACT exp(S^T/8) -> bf16 SBUF                 (the softmax exp)
  AV      = vh_ext.T @ expS  (accumulate over k tiles)  PSUM [65, 512]
            row 64 = softmax denominator (ones column)
  attnT   = (AV[0:64] * 1/denom) -> bf16                [64, 2048] per head
  y      += attnT_h.T @ wp_h  (K=64 contraction)        [2048, 1024] fp32
"""

import sys

import numpy as np

if "/opt/trn_rl_repo" not in sys.path:
    sys.path.insert(0, "/opt/trn_rl_repo")

B, T, DIN = 4, 2048, 1024
NH, HD, EMB = 16, 64, 1024
HGD = 512          # per-core qk/v dims (8 heads * 64)
NKT = DIN // 128   # 8  input-dim k tiles
NQC = T // 512     # 4  t chunks of 512
NTT = T // 128     # 16 t tiles of 128
NM = HGD // 128    # 4  head-pair m tiles
HPC = 8            # heads per core

_CACHE = {}

# build-time tunables (model-guided sweeps)
TUNE = {"CAP": 10, "EXPS_BUFS": 12, "DRAIN": 2}


def _build_nc():
    import concourse.bacc as bacc
    import concourse.bass as bass
    import concourse.mybir as mybir
    import concourse.tile as tile

    dt = mybir.dt
    AF = mybir.ActivationFunctionType

    nc = bacc.Bacc("TRN2", target_bir_lowering=False, debug=False)
    xq = nc.declare_dram_parameter("xq", [T, DIN], dt.float32, isOutput=False)
    xk = nc.declare_dram_parameter("xk", [T, DIN], dt.float32, isOutput=False)
    xv = nc.declare_dram_parameter("xv", [T, DIN], dt.float32, isOutput=False)
    wq = nc.declare_dram_parameter("wq", [DIN, HGD], dt.float32, isOutput=False)
    wk = nc.declare_dram_parameter("wk", [DIN, HGD], dt.float32, isOutput=False)
    wv = nc.declare_dram_parameter("wv", [DIN, HGD], dt.float32, isOutput=False)
    wp = nc.declare_dram_parameter("wp", [HGD, EMB], dt.float32, isOutput=False)
    y = nc.declare_dram_parameter("y", [T, EMB], dt.float32, isOutput=True)

    with tile.TileContext(nc) as tc:
        from contextlib import ExitStack

        with ExitStack() as ctx:
            p_w = ctx.enter_context(tc.tile_pool(name="weights", bufs=1))
            p_xt = ctx.enter_context(tc.tile_pool(name="xt", bufs=4))
            p_qkh = ctx.enter_context(tc.tile_pool(name="qkh", bufs=1))
            p_vh = ctx.enter_context(tc.tile_pool(name="vh", bufs=1))
            p_exps = ctx.enter_context(tc.tile_pool(name="exps", bufs=TUNE["EXPS_BUFS"]))
            p_attn = ctx.enter_context(tc.tile_pool(name="attn", bufs=1))
            p_norm = ctx.enter_context(tc.tile_pool(name="norm", bufs=4))
            p_y = ctx.enter_context(tc.tile_pool(name="ysb", bufs=2))
            p_ps = ctx.enter_context(tc.tile_pool(name="psum_s", bufs=2, space="PSUM"))
            p_av = ctx.enter_context(tc.tile_pool(name="psum_av", bufs=1, space="PSUM"))
            p_big = ctx.enter_context(tc.tile_pool(name="psum_big", bufs=2, space="PSUM"))

            # DRAM scratch used to partition-broadcast softmax denominators
            nscr = nc.dram_tensor("nscr", [32, 512], dt.float32)
            # bf16 copies of the inputs (DRAM->DRAM cast), transposed-read later
            xqb = nc.dram_tensor("xqb", [T, DIN], dt.bfloat16)
            xkb = nc.dram_tensor("xkb", [T, DIN], dt.bfloat16)
            xvb = nc.dram_tensor("xvb", [T, DIN], dt.bfloat16)

            # --- weights: cast to bf16 during SWDGE DMA, k-tiled layouts ---
            # w*_sb[p, kt, n] = w[kt*128 + p, n]
            wq_sb = p_w.tile([128, NKT, HGD], dt.bfloat16, tag="wq")
            wk_sb = p_w.tile([128, NKT, HGD], dt.bfloat16, tag="wk")
            wv_sb = p_w.tile([128, NKT, HGD], dt.bfloat16, tag="wv")
            # wp pair-tiled to match attnT: wp_sb[p, m, e] = wp[m*128+p, e]
            wp_sb = p_w.tile([128, NM, EMB], dt.bfloat16, tag="wp")


            # persistent activations
            qhT = [p_qkh.tile([128, T], dt.bfloat16, tag=f"qhT{m}", name=f"qhT{m}") for m in range(NM)]
            khT = [p_qkh.tile([128, T], dt.bfloat16, tag=f"khT{m}", name=f"khT{m}") for m in range(NM)]
            # vh_ext[t, h, 0:64] = vh, vh_ext[t, h, 64] = 1.0 (softmax denom)
            vh_ext = [p_vh.tile([128, HPC, HD + 1], dt.bfloat16, tag=f"vh{tt}", name=f"vh{tt}") for tt in range(NTT)]
            for tt in range(NTT):
                nc.vector.memset(vh_ext[tt][:, :, HD : HD + 1], 1.0)
            # attnT[m]: head 2m in rows 0:64, head 2m+1 in rows 64:128
            attnT = [p_attn.tile([128, T], dt.bfloat16, tag=f"at{m}", name=f"at{m}") for m in range(NM)]

            # --- phase 1: loads, transposes, projections (per 512-token block) ---
            # cast f32 -> bf16 into DRAM scratch (SWDGE), chunked for overlap.
            # First the block-0 casts + the weights they unblock, so the first
            # projection matmuls start as early as possible.
            tsl0 = slice(0, 512)
            nc.gpsimd.dma_start(out=xkb[tsl0, :], in_=xk[tsl0, :])
            nc.gpsimd.dma_start(out=wk_sb[:], in_=wk.rearrange("(kt p) n -> p kt n", p=128))
            nc.gpsimd.dma_start(out=xvb[tsl0, :], in_=xv[tsl0, :])
            nc.gpsimd.dma_start(out=wv_sb[:], in_=wv.rearrange("(kt p) n -> p kt n", p=128))
            nc.gpsimd.dma_start(out=xqb[tsl0, :], in_=xq[tsl0, :])
            nc.gpsimd.dma_start(out=wq_sb[:], in_=wq.rearrange("(kt p) n -> p kt n", p=128))
            for qcb in range(1, NQC):
                tsl = slice(512 * qcb, 512 * (qcb + 1))
                nc.gpsimd.dma_start(out=xkb[tsl, :], in_=xk[tsl, :])
                nc.gpsimd.dma_start(out=xvb[tsl, :], in_=xv[tsl, :])
                nc.gpsimd.dma_start(out=xqb[tsl, :], in_=xq[tsl, :])
            nc.gpsimd.dma_start(out=wp_sb[:], in_=wp.rearrange("(m p) e -> p m e", p=128))

            def load_T(xb, qcb):
                """xbar-transpose one 512-token block from bf16 DRAM.

                xt[p, kt, t] = x[512*qcb + t, kt*128 + p]

                The XPOSE ISA instruction has a single semaphore-wait slot, so
                a tiny ordinary DMA first touches both the source chunk and the
                whole destination tile; it absorbs the source-ready and
                slot-WAR waits, leaving <=1 wait for the transpose itself.
                """
                xt = p_xt.tile([128, NKT, 512], dt.bfloat16, tag="xt")
                row = xb[512 * qcb : 512 * qcb + 1, 0:NKT]
                nc.sync.dma_start(
                    out=xt[:, :, 0:1], in_=row.to_broadcast([128, NKT])
                )
                nc.sync.dma_start(
                    out=xt[:], in_=xb[512 * qcb : 512 * (qcb + 1), :], transpose=True
                )
                return xt

            # ---- attention emission state (interleaved with phase 1) ----
            # Window = (qc, pair): 2 heads x 512 queries, accumulated over 16
            # key tiles. Only one window owns the AV PSUM accumulators at a
            # time; later windows run S+exp ahead into SBUF slots (lookahead
            # bounded by the exps pool) so the scalar engine never idles.
            windows = [(qc, pair) for qc in range(NQC) for pair in range(NM)]
            sdone = {w: 0 for w in windows}
            buf = {w: [] for w in windows}
            av_tiles = {}
            state = {"open": 0, "inflight": 0}
            CAP = TUNE["CAP"]

            def emit_s_exp(w):
                qc, pair = w
                kt = sdone[w]
                qsl_w = slice(512 * qc, 512 * (qc + 1))
                ksl = slice(128 * kt, 128 * (kt + 1))
                ps = p_ps.tile([128, 1024], dt.float32, tag="pss", name="pss")
                nc.tensor.matmul(
                    ps[:, 0:512], khT[pair][0:64, ksl], qhT[pair][0:64, qsl_w],
                    start=True, stop=True,
                )
                nc.tensor.matmul(
                    ps[:, 512:1024], khT[pair][64:128, ksl], qhT[pair][64:128, qsl_w],
                    start=True, stop=True,
                )
                es = p_exps.tile([128, 1024], dt.bfloat16, tag="es", name="es")
                nc.scalar.activation(es[:], ps[:], AF.Exp, scale=1.0 / 8.0)
                buf[w].append((kt, es))
                sdone[w] += 1
                state["inflight"] += 1

            def emit_av_drain(w):
                qc, pair = w
                if w not in av_tiles:
                    av_a = p_av.tile([HD + 1, 512], dt.float32, tag="ava", name="ava")
                    av_b = p_av.tile([HD + 1, 512], dt.float32, tag="avb", name="avb")
                    av_tiles[w] = (av_a, av_b)
                av_a, av_b = av_tiles[w]
                for kt, es in buf[w]:
                    nc.tensor.matmul(
                        av_a[:], vh_ext[kt][:, 2 * pair, :], es[:, 0:512],
                        start=(kt == 0), stop=(kt == NTT - 1),
                    )
                    nc.tensor.matmul(
                        av_b[:], vh_ext[kt][:, 2 * pair + 1, :], es[:, 512:1024],
                        start=(kt == 0), stop=(kt == NTT - 1),
                    )
                    state["inflight"] -= 1
                buf[w].clear()

            def emit_norm(w):
                qc, pair = w
                qsl_w = slice(512 * qc, 512 * (qc + 1))
                av_a, av_b = av_tiles.pop(w)
                for h2, av in ((0, av_a), (1, av_b)):
                    i = (pair * NQC + qc) * 2 + h2
                    # evacuate the accumulator to SBUF so the PSUM bank frees
                    # immediately; normalize off the staged copy
                    st = p_norm.tile([HD + 1, 512], dt.float32, tag=f"st{h2}", name="st")
                    nc.vector.tensor_copy(st[:], av[:])
                    rc = p_norm.tile([1, 512], dt.float32, tag="rc", name="rc")
                    nc.vector.reciprocal(rc[:], st[HD : HD + 1, :])
                    nc.sync.dma_start(out=nscr[i : i + 1, :], in_=rc[:])
                    rb = p_norm.tile([64, 512], dt.float32, tag="rb", name="rb")
                    nc.sync.dma_start(
                        out=rb[:], in_=nscr[i : i + 1, :].to_broadcast([64, 512])
                    )
                    nc.vector.tensor_mul(
                        attnT[pair][64 * h2 : 64 * h2 + 64, qsl_w], st[0:HD, :], rb[:]
                    )

            def emit_proj_qc(qc):
                for tt in range(4 * qc, 4 * qc + 4):
                    tsl = slice(128 * tt, 128 * (tt + 1))
                    for ec in range(2):
                        esl = slice(512 * ec, 512 * (ec + 1))
                        ps = p_big.tile([128, 512], dt.float32, tag="psb", name="psb")
                        for m in range(NM):
                            nc.tensor.matmul(
                                ps[:],
                                attnT[m][:, tsl],
                                wp_sb[:, m, esl],
                                start=(m == 0),
                                stop=(m == NM - 1),
                            )
                        ysb = p_y.tile([128, 512], dt.float32, tag="ysb", name="ysb")
                        nc.vector.tensor_copy(ysb[:], ps[:])
                        nc.sync.dma_start(out=y[tsl, esl], in_=ysb[:])

            def emit_attn_progress(hi):
                # advance the open window as far as data allows
                while state["open"] < len(windows):
                    w = windows[state["open"]]
                    while sdone[w] < hi:
                        emit_s_exp(w)
                        if len(buf[w]) >= TUNE["DRAIN"]:
                            emit_av_drain(w)
                    emit_av_drain(w)
                    if sdone[w] == NTT:
                        emit_norm(w)
                        qc, pair = w
                        state["open"] += 1
                        if pair == NM - 1:
                            emit_proj_qc(qc)
                    else:
                        break
                # lookahead S+exp into free slots
                li = state["open"] + 1
                while state["inflight"] < CAP and li < len(windows):
                    w2 = windows[li]
                    if sdone[w2] < hi:
                        emit_s_exp(w2)
                    else:
                        li += 1

            for qcb in range(NQC):
                xkT = load_T(xkb, qcb)
                xvT = load_T(xvb, qcb)
                xqT = load_T(xqb, qcb)
                qsl = slice(512 * qcb, 512 * (qcb + 1))
                # K projection: khT[m][:, qsl] = wk[:, m].T @ xk[qsl].T
                for m in range(NM):
                    ps = p_big.tile([128, 512], dt.float32, tag="psb")
                    for kt in range(NKT):
                        nc.tensor.matmul(
                            ps[:],
                            wk_sb[:, kt, 128 * m : 128 * (m + 1)],
                            xkT[:, kt, :],
                            start=(kt == 0),
                            stop=(kt == NKT - 1),
                        )
                    nc.vector.tensor_copy(khT[m][:, qsl], ps[:])
                # V projection: vh[tt] = xv[tt] @ wv   (natural layout)
                for ti in range(4):
                    tt = 4 * qcb + ti
                    ps = p_big.tile([128, 512], dt.float32, tag="psb")
                    for kt in range(NKT):
                        nc.tensor.matmul(
                            ps[:],
                            xvT[:, kt, 128 * ti : 128 * (ti + 1)],
                            wv_sb[:, kt, :],
                            start=(kt == 0),
                            stop=(kt == NKT - 1),
                        )
                    nc.vector.tensor_copy(
                        vh_ext[tt][:, :, 0:HD],
                        ps.rearrange("p (h d) -> p h d", h=HPC),
                    )
                # Q projection
                for m in range(NM):
                    ps = p_big.tile([128, 512], dt.float32, tag="psb")
                    for kt in range(NKT):
                        nc.tensor.matmul(
                            ps[:],
                            wq_sb[:, kt, 128 * m : 128 * (m + 1)],
                            xqT[:, kt, :],
                            start=(kt == 0),
                            stop=(kt == NKT - 1),
                        )
                    nc.vector.tensor_copy(qhT[m][:, qsl], ps[:])

                emit_attn_progress(4 * (qcb + 1))

    nc.compile()
    return nc


def _get_nc():
    if "nc" not in _CACHE:
        _CACHE["nc"] = _build_nc()
    return _CACHE["nc"]


def core_input_map(k, q, v, w_key, w_query, w_value, w_proj, core):
    b, g = core // 2, core % 2
    sl = slice(g * HGD, (g + 1) * HGD)
    f32 = np.float32
    return {
        "xq": np.ascontiguousarray(q[b], dtype=f32),
        "xk": np.ascontiguousarray(k[b], dtype=f32),
        "xv": np.ascontiguousarray(v[b], dtype=f32),
        "wq": np.ascontiguousarray(w_query[:, sl], dtype=f32),
        "wk": np.ascontiguousarray(w_key[:, sl], dtype=f32),
        "wv": np.ascontiguousarray(w_value[:, sl], dtype=f32),
        "wp": np.ascontiguousarray(w_proj[sl, :], dtype=f32),
    }


def kernel(k, q, v, w_key, w_query, w_value, w_proj):
    from concourse.bass_utils import run_bass_kernel_spmd

    nc = _get_nc()
    in_maps = [
        core_input_map(k, q, v, w_key, w_query, w_value, w_proj, c) for c in range(8)
    ]
    res = run_bass_kernel_spmd(nc, in_maps, list(range(8))).results
    out = np.empty((B, T, EMB), np.float32)
    for b in range(B):
        out[b] = res[2 * b]["y"] + res[2 * b + 1]["y"]
    return out


# revision 29
# speedup vs baseline: 270.9130x; 270.9130x over previous
"""Multi-head attention kernel for Trainium2, 8 NeuronCores.

Problem: B=4, T=2048, D_in=1024, 16 heads x 64 dim, E=1024 (fp32).

Sharding: (batch x head-group). Core c handles batch b=c//2 and head-group
g=c%2 (8 heads, 512 qk/v dims). Each core computes its batch's QKV
projections restricted to its heads, full attention for those heads, and a
partial output projection. The host sums the two partial projections per
batch (the only cross-core reduction) and stacks batches.

Per-core dataflow (all matmuls bf16 inputs, fp32 PSUM accumulation):
  xT      = dma-xbar-transpose(cast_bf16(x))            [1024, 2048] per tensor
  qhT/khT = w.T @ xT   (weights stationary)             [512, 2048]
  vh      = xT.T @ wv  (x stationary) + ones column     [2048, 8*65]
  S^T     = khT_h.T @ qhT_h per head pair               PSUM [128,1024]
  expS    = ACT exp(S^T/8) -> bf16 SBUF                 (the softmax exp)
  AV      = vh_ext.T @ expS  (accumulate over k tiles)  PSUM [65, 512]
            row 64 = softmax denominator (ones column)
  attnT   = (AV[0:64] * 1/denom) -> bf16                [64, 2048] per head
  y      += attnT_h.T @ wp_h  (K=64 contraction)        [2048, 1024] fp32
"""

import sys

import numpy as np

if "/opt/trn_rl_repo" not in sys.path:
    sys.path.insert(0, "/opt/trn_rl_repo")

B, T, DIN = 4, 2048, 1024
NH, HD, EMB = 16, 64, 1024
HGD = 512          # per-core qk/v dims (8 heads * 64)
NKT = DIN // 128   # 8  input-dim k tiles
NQC = T // 512     # 4  t chunks of 512
NTT = T // 128     # 16 t tiles of 128
NM = HGD // 128    # 4  head-pair m tiles
HPC = 8            # heads per core

_CACHE = {}

# build-time tunables (model-guided sweeps)
TUNE = {"CAP": 10, "EXPS_BUFS": 12, "DRAIN": 8}


def _build_nc():
    import concourse.bacc as bacc
    import concourse.bass as bass
    import concourse.mybir as mybir
    import concourse.tile as tile

    dt = mybir.dt
    AF = mybir.ActivationFunctionType

    nc = bacc.Bacc("TRN2", target_bir_lowering=False, debug=False)
    xq = nc.declare_dram_parameter("xq", [T, DIN], dt.float32, isOutput=False)
    xk = nc.declare_dram_parameter("xk", [T, DIN], dt.float32, isOutput=False)
    xv = nc.declare_dram_parameter("xv", [T, DIN], dt.float32, isOutput=False)
    wq = nc.declare_dram_parameter("wq", [DIN, HGD], dt.float32, isOutput=False)
    wk = nc.declare_dram_parameter("wk", [DIN, HGD], dt.float32, isOutput=False)
    wv = nc.declare_dram_parameter("wv", [DIN, HGD], dt.float32, isOutput=False)
    wp = nc.declare_dram_parameter("wp", [HGD, EMB], dt.float32, isOutput=False)
    y = nc.declare_dram_parameter("y", [T, EMB], dt.float32, isOutput=True)

    with tile.TileContext(nc) as tc:
        from contextlib import ExitStack

        with ExitStack() as ctx:
            p_w = ctx.enter_context(tc.tile_pool(name="weights", bufs=1))
            p_xt = ctx.enter_context(tc.tile_pool(name="xt", bufs=4))
            p_qkh = ctx.enter_context(tc.tile_pool(name="qkh", bufs=1))
            p_vh = ctx.enter_context(tc.tile_pool(name="vh", bufs=1))
            p_exps = ctx.enter_context(tc.tile_pool(name="exps", bufs=TUNE["EXPS_BUFS"]))
            p_attn = ctx.enter_context(tc.tile_pool(name="attn", bufs=1))
            p_norm = ctx.enter_context(tc.tile_pool(name="norm", bufs=4))
            p_y = ctx.enter_context(tc.tile_pool(name="ysb", bufs=2))
            p_ps = ctx.enter_context(tc.tile_pool(name="psum_s", bufs=2, space="PSUM"))
            p_av = ctx.enter_context(tc.tile_pool(name="psum_av", bufs=1, space="PSUM"))
            p_big = ctx.enter_context(tc.tile_pool(name="psum_big", bufs=2, space="PSUM"))

            # DRAM scratch used to partition-broadcast softmax denominators
            nscr = nc.dram_tensor("nscr", [32, 512], dt.float32)
            # bf16 copies of the inputs (DRAM->DRAM cast), transposed-read later
            xqb = nc.dram_tensor("xqb", [T, DIN], dt.bfloat16)
            xkb = nc.dram_tensor("xkb", [T, DIN], dt.bfloat16)
            xvb = nc.dram_tensor("xvb", [T, DIN], dt.bfloat16)

            # --- weights: cast to bf16 during SWDGE DMA, k-tiled layouts ---
            # w*_sb[p, kt, n] = w[kt*128 + p, n]
            wq_sb = p_w.tile([128, NKT, HGD], dt.bfloat16, tag="wq")
            wk_sb = p_w.tile([128, NKT, HGD], dt.bfloat16, tag="wk")
            wv_sb = p_w.tile([128, NKT, HGD], dt.bfloat16, tag="wv")
            # wp pair-tiled to match attnT: wp_sb[p, m, e] = wp[m*128+p, e]
            wp_sb = p_w.tile([128, NM, EMB], dt.bfloat16, tag="wp")


            # persistent activations
            qhT = [p_qkh.tile([128, T], dt.bfloat16, tag=f"qhT{m}", name=f"qhT{m}") for m in range(NM)]
            khT = [p_qkh.tile([128, T], dt.bfloat16, tag=f"khT{m}", name=f"khT{m}") for m in range(NM)]
            # vh_ext[t, h, 0:64] = vh, vh_ext[t, h, 64] = 1.0 (softmax denom)
            vh_ext = [p_vh.tile([128, HPC, HD + 1], dt.bfloat16, tag=f"vh{tt}", name=f"vh{tt}") for tt in range(NTT)]
            for tt in range(NTT):
                nc.vector.memset(vh_ext[tt][:, :, HD : HD + 1], 1.0)
            # attnT[m]: head 2m in rows 0:64, head 2m+1 in rows 64:128
            attnT = [p_attn.tile([128, T], dt.bfloat16, tag=f"at{m}", name=f"at{m}") for m in range(NM)]

            # --- phase 1: loads, transposes, projections (per 512-token block) ---
            # cast f32 -> bf16 into DRAM scratch (SWDGE), chunked for overlap.
            # First the block-0 casts + the weights they unblock, so the first
            # projection matmuls start as early as possible.
            tsl0 = slice(0, 512)
            nc.gpsimd.dma_start(out=xkb[tsl0, :], in_=xk[tsl0, :])
            wk_r = wk.rearrange("(kt p) n -> p kt n", p=128)
            nc.gpsimd.dma_start(out=wk_sb[:, :, 0:128], in_=wk_r[:, :, 0:128])
            nc.gpsimd.dma_start(out=wk_sb[:, :, 128:HGD], in_=wk_r[:, :, 128:HGD])
            nc.gpsimd.dma_start(out=xvb[tsl0, :], in_=xv[tsl0, :])
            nc.gpsimd.dma_start(out=wv_sb[:], in_=wv.rearrange("(kt p) n -> p kt n", p=128))
            nc.gpsimd.dma_start(out=xqb[tsl0, :], in_=xq[tsl0, :])
            nc.gpsimd.dma_start(out=wq_sb[:], in_=wq.rearrange("(kt p) n -> p kt n", p=128))
            for qcb in range(1, NQC):
                tsl = slice(512 * qcb, 512 * (qcb + 1))
                nc.gpsimd.dma_start(out=xkb[tsl, :], in_=xk[tsl, :])
                nc.gpsimd.dma_start(out=xvb[tsl, :], in_=xv[tsl, :])
                nc.gpsimd.dma_start(out=xqb[tsl, :], in_=xq[tsl, :])
            nc.gpsimd.dma_start(out=wp_sb[:], in_=wp.rearrange("(m p) e -> p m e", p=128))

            def load_T(xb, qcb):
                """xbar-transpose one 512-token block from bf16 DRAM.

                xt[p, kt, t] = x[512*qcb + t, kt*128 + p]

                The XPOSE ISA instruction has a single semaphore-wait slot, so
                a tiny ordinary DMA first touches both the source chunk and the
                whole destination tile; it absorbs the source-ready and
                slot-WAR waits, leaving <=1 wait for the transpose itself.
                """
                xt = p_xt.tile([128, NKT, 512], dt.bfloat16, tag="xt")
                row = xb[512 * qcb : 512 * qcb + 1, 0:NKT]
                nc.sync.dma_start(
                    out=xt[:, :, 0:1], in_=row.to_broadcast([128, NKT])
                )
                nc.sync.dma_start(
                    out=xt[:], in_=xb[512 * qcb : 512 * (qcb + 1), :], transpose=True
                )
                return xt

            # ---- attention emission state (interleaved with phase 1) ----
            # Window = (qc, pair): 2 heads x 512 queries, accumulated over 16
            # key tiles. Only one window owns the AV PSUM accumulators at a
            # time; later windows run S+exp ahead into SBUF slots (lookahead
            # bounded by the exps pool) so the scalar engine never idles.
            windows = [(qc, pair) for qc in range(NQC) for pair in range(NM)]
            sdone = {w: 0 for w in windows}
            buf = {w: [] for w in windows}
            av_tiles = {}
            state = {"open": 0, "inflight": 0}
            CAP = TUNE["CAP"]

            def emit_s_exp(w):
                qc, pair = w
                kt = sdone[w]
                qsl_w = slice(512 * qc, 512 * (qc + 1))
                ksl = slice(128 * kt, 128 * (kt + 1))
                ps = p_ps.tile([128, 1024], dt.float32, tag="pss", name="pss")
                nc.tensor.matmul(
                    ps[:, 0:512], khT[pair][0:64, ksl], qhT[pair][0:64, qsl_w],
                    start=True, stop=True,
                )
                nc.tensor.matmul(
                    ps[:, 512:1024], khT[pair][64:128, ksl], qhT[pair][64:128, qsl_w],
                    start=True, stop=True,
                )
                es = p_exps.tile([128, 1024], dt.bfloat16, tag="es", name="es")
                nc.scalar.activation(es[:], ps[:], AF.Exp, scale=1.0 / 8.0)
                buf[w].append((kt, es))
                sdone[w] += 1
                state["inflight"] += 1

            def emit_av_drain(w):
                qc, pair = w
                if w not in av_tiles:
                    av_a = p_av.tile([HD + 1, 512], dt.float32, tag="ava", name="ava")
                    av_b = p_av.tile([HD + 1, 512], dt.float32, tag="avb", name="avb")
                    av_tiles[w] = (av_a, av_b)
                av_a, av_b = av_tiles[w]
                for kt, es in buf[w]:
                    nc.tensor.matmul(
                        av_a[:], vh_ext[kt][:, 2 * pair, :], es[:, 0:512],
                        start=(kt == 0), stop=(kt == NTT - 1),
                    )
                    nc.tensor.matmul(
                        av_b[:], vh_ext[kt][:, 2 * pair + 1, :], es[:, 512:1024],
                        start=(kt == 0), stop=(kt == NTT - 1),
                    )
                    state["inflight"] -= 1
                buf[w].clear()

            def emit_norm(w):
                qc, pair = w
                qsl_w = slice(512 * qc, 512 * (qc + 1))
                av_a, av_b = av_tiles.pop(w)
                for h2, av in ((0, av_a), (1, av_b)):
                    i = (pair * NQC + qc) * 2 + h2
                    # evacuate the accumulator to SBUF so the PSUM bank frees
                    # immediately; normalize off the staged copy
                    st = p_norm.tile([HD + 1, 512], dt.float32, tag=f"st{h2}", name="st")
                    nc.vector.tensor_copy(st[:], av[:])
                    rc = p_norm.tile([1, 512], dt.float32, tag="rc", name="rc")
                    nc.vector.reciprocal(rc[:], st[HD : HD + 1, :])
                    nc.sync.dma_start(out=nscr[i : i + 1, :], in_=rc[:])
                    rb = p_norm.tile([64, 512], dt.float32, tag="rb", name="rb")
                    nc.sync.dma_start(
                        out=rb[:], in_=nscr[i : i + 1, :].to_broadcast([64, 512])
                    )
                    nc.vector.tensor_mul(
                        attnT[pair][64 * h2 : 64 * h2 + 64, qsl_w], st[0:HD, :], rb[:]
                    )

            def emit_proj_qc(qc):
                for tt in range(4 * qc, 4 * qc + 4):
                    tsl = slice(128 * tt, 128 * (tt + 1))
                    for ec in range(2):
                        esl = slice(512 * ec, 512 * (ec + 1))
                        ps = p_big.tile([128, 512], dt.float32, tag="psb", name="psb")
                        for m in range(NM):
                            nc.tensor.matmul(
                                ps[:],
                                attnT[m][:, tsl],
                                wp_sb[:, m, esl],
                                start=(m == 0),
                                stop=(m == NM - 1),
                            )
                        ysb = p_y.tile([128, 512], dt.float32, tag="ysb", name="ysb")
                        nc.vector.tensor_copy(ysb[:], ps[:])
                        nc.sync.dma_start(out=y[tsl, esl], in_=ysb[:])

            def emit_attn_progress(hi):
                # advance the open window as far as data allows
                while state["open"] < len(windows):
                    w = windows[state["open"]]
                    while sdone[w] < hi:
                        emit_s_exp(w)
                        if len(buf[w]) >= TUNE["DRAIN"]:
                            emit_av_drain(w)
                    emit_av_drain(w)
                    if sdone[w] == NTT:
                        emit_norm(w)
                        qc, pair = w
                        state["open"] += 1
                        if pair == NM - 1:
                            emit_proj_qc(qc)
                    else:
                        break
                # lookahead S+exp into free slots
                li = state["open"] + 1
                while state["inflight"] < CAP and li < len(windows):
                    w2 = windows[li]
                    if sdone[w2] < hi:
                        emit_s_exp(w2)
                    else:
                        li += 1

            for qcb in range(NQC):
                xkT = load_T(xkb, qcb)
                xvT = load_T(xvb, qcb)
                xqT = load_T(xqb, qcb)
                qsl = slice(512 * qcb, 512 * (qcb + 1))
                # K projection: khT[m][:, qsl] = wk[:, m].T @ xk[qsl].T
                for m in range(NM):
                    ps = p_big.tile([128, 512], dt.float32, tag="psb")
                    for kt in range(NKT):
                        nc.tensor.matmul(
                            ps[:],
                            wk_sb[:, kt, 128 * m : 128 * (m + 1)],
                            xkT[:, kt, :],
                            start=(kt == 0),
                            stop=(kt == NKT - 1),
                        )
                    nc.vector.tensor_copy(khT[m][:, qsl], ps[:])
                # V projection: vh[tt] = xv[tt] @ wv   (natural layout)
                for ti in range(4):
                    tt = 4 * qcb + ti
                    ps = p_big.tile([128, 512], dt.float32, tag="psb")
                    for kt in range(NKT):
                        nc.tensor.matmul(
                            ps[:],
                            xvT[:, kt, 128 * ti : 128 * (ti + 1)],
                            wv_sb[:, kt, :],
                            start=(kt == 0),
                            stop=(kt == NKT - 1),
                        )
                    nc.vector.tensor_copy(
                        vh_ext[tt][:, :, 0:HD],
                        ps.rearrange("p (h d) -> p h d", h=HPC),
                    )
                # Q projection
                for m in range(NM):
                    ps = p_big.tile([128, 512], dt.float32, tag="psb")
                    for kt in range(NKT):
                        nc.tensor.matmul(
                            ps[:],
                            wq_sb[:, kt, 128 * m : 128 * (m + 1)],
                            xqT[:, kt, :],
                            start=(kt == 0),
                            stop=(kt == NKT - 1),
                        )
                    nc.vector.tensor_copy(qhT[m][:, qsl], ps[:])

                emit_attn_progress(4 * (qcb + 1))

    nc.compile()
    return nc


def _get_nc():
    if "nc" not in _CACHE:
        _CACHE["nc"] = _build_nc()
    return _CACHE["nc"]


def core_input_map(k, q, v, w_key, w_query, w_value, w_proj, core):
    b, g = core // 2, core % 2
    sl = slice(g * HGD, (g + 1) * HGD)
    f32 = np.float32
    return {
        "xq": np.ascontiguousarray(q[b], dtype=f32),
        "xk": np.ascontiguousarray(k[b], dtype=f32),
        "xv": np.ascontiguousarray(v[b], dtype=f32),
        "wq": np.ascontiguousarray(w_query[:, sl], dtype=f32),
        "wk": np.ascontiguousarray(w_key[:, sl], dtype=f32),
        "wv": np.ascontiguousarray(w_value[:, sl], dtype=f32),
        "wp": np.ascontiguousarray(w_proj[sl, :], dtype=f32),
    }


def kernel(k, q, v, w_key, w_query, w_value, w_proj):
    from concourse.bass_utils import run_bass_kernel_spmd

    nc = _get_nc()
    in_maps = [
        core_input_map(k, q, v, w_key, w_query, w_value, w_proj, c) for c in range(8)
    ]
    res = run_bass_kernel_spmd(nc, in_maps, list(range(8))).results
    out = np.empty((B, T, EMB), np.float32)
    for b in range(B):
        out[b] = res[2 * b]["y"] + res[2 * b + 1]["y"]
    return out


# revision 34
# speedup vs baseline: 278.8839x; 1.0294x over previous
"""Multi-head attention kernel for Trainium2, 8 NeuronCores.

Problem: B=4, T=2048, D_in=1024, 16 heads x 64 dim, E=1024 (fp32).

Sharding: (batch x head-group). Core c handles batch b=c//2 and head-group
g=c%2 (8 heads, 512 qk/v dims). Each core computes its batch's QKV
projections restricted to its heads, full attention for those heads, and a
partial output projection. The host sums the two partial projections per
batch (the only cross-core reduction) and stacks batches.

Per-core dataflow (all matmuls bf16 inputs, fp32 PSUM accumulation):
  xT      = dma-xbar-transpose(cast_bf16(x))            [1024, 2048] per tensor
  qhT/khT = w.T @ xT   (weights stationary)             [512, 2048]
  vh      = xT.T @ wv  (x stationary) + ones column     [2048, 8*65]
  S^T     = khT_h.T @ qhT_h per head pair               PSUM [128,1024]
  expS    = ACT exp(S^T/8) -> bf16 SBUF                 (the softmax exp)
  AV      = vh_ext.T @ expS  (accumulate over k tiles)  PSUM [65, 512]
            row 64 = softmax denominator (ones column)
  attnT   = (AV[0:64] * 1/denom) -> bf16                [64, 2048] per head
  y      += attnT_h.T @ wp_h  (K=64 contraction)        [2048, 1024] fp32
"""

import sys

import numpy as np

if "/opt/trn_rl_repo" not in sys.path:
    sys.path.insert(0, "/opt/trn_rl_repo")

B, T, DIN = 4, 2048, 1024
NH, HD, EMB = 16, 64, 1024
HGD = 512          # per-core qk/v dims (8 heads * 64)
NKT = DIN // 128   # 8  input-dim k tiles
NQC = T // 512     # 4  t chunks of 512
NTT = T // 128     # 16 t tiles of 128
NM = HGD // 128    # 4  head-pair m tiles
HPC = 8            # heads per core

_CACHE = {}

# build-time tunables (model-guided sweeps)
TUNE = {"CAP": 10, "EXPS_BUFS": 20, "DRAIN": 16}


def _build_nc():
    import concourse.bacc as bacc
    import concourse.bass as bass
    import concourse.mybir as mybir
    import concourse.tile as tile

    dt = mybir.dt
    AF = mybir.ActivationFunctionType

    nc = bacc.Bacc("TRN2", target_bir_lowering=False, debug=False)
    xq = nc.declare_dram_parameter("xq", [T, DIN], dt.float32, isOutput=False)
    xk = nc.declare_dram_parameter("xk", [T, DIN], dt.float32, isOutput=False)
    xv = nc.declare_dram_parameter("xv", [T, DIN], dt.float32, isOutput=False)
    wq = nc.declare_dram_parameter("wq", [DIN, HGD], dt.float32, isOutput=False)
    wk = nc.declare_dram_parameter("wk", [DIN, HGD], dt.float32, isOutput=False)
    wv = nc.declare_dram_parameter("wv", [DIN, HGD], dt.float32, isOutput=False)
    wp = nc.declare_dram_parameter("wp", [HGD, EMB], dt.float32, isOutput=False)
    y = nc.declare_dram_parameter("y", [T, EMB], dt.float32, isOutput=True)

    with tile.TileContext(nc) as tc:
        from contextlib import ExitStack

        with ExitStack() as ctx:
            p_w = ctx.enter_context(tc.tile_pool(name="weights", bufs=1))
            p_xt = ctx.enter_context(tc.tile_pool(name="xt", bufs=4))
            p_qkh = ctx.enter_context(tc.tile_pool(name="qkh", bufs=1))
            p_vh = ctx.enter_context(tc.tile_pool(name="vh", bufs=1))
            p_exps = ctx.enter_context(tc.tile_pool(name="exps", bufs=TUNE["EXPS_BUFS"]))
            p_attn = ctx.enter_context(tc.tile_pool(name="attn", bufs=1))
            p_norm = ctx.enter_context(tc.tile_pool(name="norm", bufs=4))
            p_y = ctx.enter_context(tc.tile_pool(name="ysb", bufs=2))
            p_ps = ctx.enter_context(tc.tile_pool(name="psum_s", bufs=2, space="PSUM"))
            p_av = ctx.enter_context(tc.tile_pool(name="psum_av", bufs=1, space="PSUM"))
            p_big = ctx.enter_context(tc.tile_pool(name="psum_big", bufs=2, space="PSUM"))

            # DRAM scratch used to partition-broadcast softmax denominators
            nscr = nc.dram_tensor("nscr", [32, 512], dt.float32)
            # bf16 copies of the inputs (DRAM->DRAM cast), transposed-read later
            xqb = nc.dram_tensor("xqb", [T, DIN], dt.bfloat16)
            xkb = nc.dram_tensor("xkb", [T, DIN], dt.bfloat16)
            xvb = nc.dram_tensor("xvb", [T, DIN], dt.bfloat16)

            # --- weights: cast to bf16 during SWDGE DMA, k-tiled layouts ---
            # w*_sb[p, kt, n] = w[kt*128 + p, n]
            wq_sb = p_w.tile([128, NKT, HGD], dt.bfloat16, tag="wq")
            wk_sb = p_w.tile([128, NKT, HGD], dt.bfloat16, tag="wk")
            wv_sb = p_w.tile([128, NKT, HGD], dt.bfloat16, tag="wv")
            # wp pair-tiled to match attnT: wp_sb[p, m, e] = wp[m*128+p, e]
            wp_sb = p_w.tile([128, NM, EMB], dt.bfloat16, tag="wp")


            # persistent activations
            qhT = [p_qkh.tile([128, T], dt.bfloat16, tag=f"qhT{m}", name=f"qhT{m}") for m in range(NM)]
            khT = [p_qkh.tile([128, T], dt.bfloat16, tag=f"khT{m}", name=f"khT{m}") for m in range(NM)]
            # vh_ext[t, h, 0:64] = vh, vh_ext[t, h, 64] = 1.0 (softmax denom)
            vh_ext = [p_vh.tile([128, HPC, HD + 1], dt.bfloat16, tag=f"vh{tt}", name=f"vh{tt}") for tt in range(NTT)]
            for tt in range(NTT):
                nc.vector.memset(vh_ext[tt][:, :, HD : HD + 1], 1.0)
            # attnT[m]: head 2m in rows 0:64, head 2m+1 in rows 64:128
            attnT = [p_attn.tile([128, T], dt.bfloat16, tag=f"at{m}", name=f"at{m}") for m in range(NM)]

            # --- phase 1: loads, transposes, projections (per 512-token block) ---
            # cast f32 -> bf16 into DRAM scratch (SWDGE), chunked for overlap.
            # First the block-0 casts + the weights they unblock, so the first
            # projection matmuls start as early as possible.
            tsl0 = slice(0, 512)
            nc.gpsimd.dma_start(out=xkb[tsl0, :], in_=xk[tsl0, :])
            wk_r = wk.rearrange("(kt p) n -> p kt n", p=128)
            nc.gpsimd.dma_start(out=wk_sb[:, :, 0:128], in_=wk_r[:, :, 0:128])
            nc.gpsimd.dma_start(out=wk_sb[:, :, 128:HGD], in_=wk_r[:, :, 128:HGD])
            nc.gpsimd.dma_start(out=xvb[tsl0, :], in_=xv[tsl0, :])
            nc.gpsimd.dma_start(out=wv_sb[:], in_=wv.rearrange("(kt p) n -> p kt n", p=128))
            nc.gpsimd.dma_start(out=xqb[tsl0, :], in_=xq[tsl0, :])
            nc.gpsimd.dma_start(out=wq_sb[:], in_=wq.rearrange("(kt p) n -> p kt n", p=128))
            for qcb in range(1, NQC):
                tsl = slice(512 * qcb, 512 * (qcb + 1))
                nc.gpsimd.dma_start(out=xkb[tsl, :], in_=xk[tsl, :])
                nc.gpsimd.dma_start(out=xvb[tsl, :], in_=xv[tsl, :])
                nc.gpsimd.dma_start(out=xqb[tsl, :], in_=xq[tsl, :])
            nc.gpsimd.dma_start(out=wp_sb[:], in_=wp.rearrange("(m p) e -> p m e", p=128))

            n_load_T = [0]

            def load_T(xb, qcb):
                """xbar-transpose one 512-token block from bf16 DRAM.

                xt[p, kt, t] = x[512*qcb + t, kt*128 + p]

                The XPOSE ISA instruction has a single semaphore-wait slot.
                Fresh pool slots only wait on the source cast (1 wait, fine);
                reused slots would also carry a WAR wait, so for those a tiny
                ordinary DMA first touches the source chunk and the whole
                destination tile, absorbing both waits.
                """
                xt = p_xt.tile([128, NKT, 512], dt.bfloat16, tag="xt")
                if n_load_T[0] >= 4:  # p_xt bufs exhausted -> slot reuse
                    row = xb[512 * qcb : 512 * qcb + 1, 0:NKT]
                    nc.sync.dma_start(
                        out=xt[:, :, 0:1], in_=row.to_broadcast([128, NKT])
                    )
                n_load_T[0] += 1
                nc.sync.dma_start(
                    out=xt[:], in_=xb[512 * qcb : 512 * (qcb + 1), :], transpose=True
                )
                return xt

            # ---- attention emission state (interleaved with phase 1) ----
            # Window = (qc, pair): 2 heads x 512 queries, accumulated over 16
            # key tiles. Only one window owns the AV PSUM accumulators at a
            # time; later windows run S+exp ahead into SBUF slots (lookahead
            # bounded by the exps pool) so the scalar engine never idles.
            windows = [(qc, pair) for qc in range(NQC) for pair in range(NM)]
            sdone = {w: 0 for w in windows}
            buf = {w: [] for w in windows}
            av_tiles = {}
            state = {"open": 0, "inflight": 0}
            CAP = TUNE["CAP"]

            def emit_s_exp(w):
                qc, pair = w
                kt = sdone[w]
                qsl_w = slice(512 * qc, 512 * (qc + 1))
                ksl = slice(128 * kt, 128 * (kt + 1))
                ps = p_ps.tile([128, 1024], dt.float32, tag="pss", name="pss")
                nc.tensor.matmul(
                    ps[:, 0:512], khT[pair][0:64, ksl], qhT[pair][0:64, qsl_w],
                    start=True, stop=True,
                )
                nc.tensor.matmul(
                    ps[:, 512:1024], khT[pair][64:128, ksl], qhT[pair][64:128, qsl_w],
                    start=True, stop=True,
                )
                es = p_exps.tile([128, 1024], dt.bfloat16, tag="es", name="es")
                nc.scalar.activation(es[:], ps[:], AF.Exp, scale=1.0 / 8.0)
                buf[w].append((kt, es))
                sdone[w] += 1
                state["inflight"] += 1

            def emit_av_drain(w):
                qc, pair = w
                if w not in av_tiles:
                    av_a = p_av.tile([HD + 1, 512], dt.float32, tag="ava", name="ava")
                    av_b = p_av.tile([HD + 1, 512], dt.float32, tag="avb", name="avb")
                    av_tiles[w] = (av_a, av_b)
                av_a, av_b = av_tiles[w]
                for kt, es in buf[w]:
                    nc.tensor.matmul(
                        av_a[:], vh_ext[kt][:, 2 * pair, :], es[:, 0:512],
                        start=(kt == 0), stop=(kt == NTT - 1),
                    )
                    nc.tensor.matmul(
                        av_b[:], vh_ext[kt][:, 2 * pair + 1, :], es[:, 512:1024],
                        start=(kt == 0), stop=(kt == NTT - 1),
                    )
                    state["inflight"] -= 1
                buf[w].clear()

            def emit_norm(w):
                qc, pair = w
                qsl_w = slice(512 * qc, 512 * (qc + 1))
                av_a, av_b = av_tiles.pop(w)
                for h2, av in ((0, av_a), (1, av_b)):
                    i = (pair * NQC + qc) * 2 + h2
                    # evacuate the accumulator to SBUF so the PSUM bank frees
                    # immediately; normalize off the staged copy
                    st = p_norm.tile([HD + 1, 512], dt.float32, tag=f"st{h2}", name="st")
                    nc.vector.tensor_copy(st[:], av[:])
                    rc = p_norm.tile([1, 512], dt.float32, tag="rc", name="rc")
                    nc.vector.reciprocal(rc[:], st[HD : HD + 1, :])
                    nc.sync.dma_start(out=nscr[i : i + 1, :], in_=rc[:])
                    rb = p_norm.tile([64, 512], dt.float32, tag="rb", name="rb")
                    nc.sync.dma_start(
                        out=rb[:], in_=nscr[i : i + 1, :].to_broadcast([64, 512])
                    )
                    nc.vector.tensor_mul(
                        attnT[pair][64 * h2 : 64 * h2 + 64, qsl_w], st[0:HD, :], rb[:]
                    )

            def emit_proj_qc(qc):
                for tt in range(4 * qc, 4 * qc + 4):
                    tsl = slice(128 * tt, 128 * (tt + 1))
                    for ec in range(2):
                        esl = slice(512 * ec, 512 * (ec + 1))
                        ps = p_big.tile([128, 512], dt.float32, tag="psb", name="psb")
                        for m in range(NM):
                            nc.tensor.matmul(
                                ps[:],
                                attnT[m][:, tsl],
                                wp_sb[:, m, esl],
                                start=(m == 0),
                                stop=(m == NM - 1),
                            )
                        ysb = p_y.tile([128, 512], dt.float32, tag="ysb", name="ysb")
                        nc.vector.tensor_copy(ysb[:], ps[:])
                        nc.sync.dma_start(out=y[tsl, esl], in_=ysb[:])

            def emit_attn_progress(hi):
                # advance the open window as far as data allows
                while state["open"] < len(windows):
                    w = windows[state["open"]]
                    while sdone[w] < hi:
                        emit_s_exp(w)
                        if len(buf[w]) >= TUNE["DRAIN"]:
                            emit_av_drain(w)
                    emit_av_drain(w)
                    if sdone[w] == NTT:
                        emit_norm(w)
                        qc, pair = w
                        state["open"] += 1
                        if pair == NM - 1:
                            emit_proj_qc(qc)
                    else:
                        break
                # lookahead S+exp into free slots
                li = state["open"] + 1
                while state["inflight"] < CAP and li < len(windows):
                    w2 = windows[li]
                    if sdone[w2] < hi:
                        emit_s_exp(w2)
                    else:
                        li += 1

            for qcb in range(NQC):
                xkT = load_T(xkb, qcb)
                xvT = load_T(xvb, qcb)
                xqT = load_T(xqb, qcb)
                qsl = slice(512 * qcb, 512 * (qcb + 1))

                def pk(m):
                    ps = p_big.tile([128, 512], dt.float32, tag="psb", name="psb")
                    for kt in range(NKT):
                        nc.tensor.matmul(
                            ps[:],
                            wk_sb[:, kt, 128 * m : 128 * (m + 1)],
                            xkT[:, kt, :],
                            start=(kt == 0),
                            stop=(kt == NKT - 1),
                        )
                    nc.vector.tensor_copy(khT[m][:, qsl], ps[:])

                def pv(ti):
                    tt = 4 * qcb + ti
                    ps = p_big.tile([128, 512], dt.float32, tag="psb", name="psb")
                    for kt in range(NKT):
                        nc.tensor.matmul(
                            ps[:],
                            xvT[:, kt, 128 * ti : 128 * (ti + 1)],
                            wv_sb[:, kt, :],
                            start=(kt == 0),
                            stop=(kt == NKT - 1),
                        )
                    nc.vector.tensor_copy(
                        vh_ext[tt][:, :, 0:HD],
                        ps.rearrange("p (h d) -> p h d", h=HPC),
                    )

                def pq(m):
                    ps = p_big.tile([128, 512], dt.float32, tag="psb", name="psb")
                    for kt in range(NKT):
                        nc.tensor.matmul(
                            ps[:],
                            wq_sb[:, kt, 128 * m : 128 * (m + 1)],
                            xqT[:, kt, :],
                            start=(kt == 0),
                            stop=(kt == NKT - 1),
                        )
                    nc.vector.tensor_copy(qhT[m][:, qsl], ps[:])

                for i in range(4):
                    pk(i)
                for i in range(4):
                    pv(i)
                for i in range(4):
                    pq(i)
                emit_attn_progress(4 * (qcb + 1))

    nc.compile()
    return nc


def _get_nc():
    if "nc" not in _CACHE:
        _CACHE["nc"] = _build_nc()
    return _CACHE["nc"]


def core_input_map(k, q, v, w_key, w_query, w_value, w_proj, core):
    b, g = core // 2, core % 2
    sl = slice(g * HGD, (g + 1) * HGD)
    f32 = np.float32
    return {
        "xq": np.ascontiguousarray(q[b], dtype=f32),
        "xk": np.ascontiguousarray(k[b], dtype=f32),
        "xv": np.ascontiguousarray(v[b], dtype=f32),
        "wq": np.ascontiguousarray(w_query[:, sl], dtype=f32),
        "wk": np.ascontiguousarray(w_key[:, sl], dtype=f32),
        "wv": np.ascontiguousarray(w_value[:, sl], dtype=f32),
        "wp": np.ascontiguousarray(w_proj[sl, :], dtype=f32),
    }


def kernel(k, q, v, w_key, w_query, w_value, w_proj):
    from concourse.bass_utils import run_bass_kernel_spmd

    nc = _get_nc()
    in_maps = [
        core_input_map(k, q, v, w_key, w_query, w_value, w_proj, c) for c in range(8)
    ]
    res = run_bass_kernel_spmd(nc, in_maps, list(range(8))).results
    out = np.empty((B, T, EMB), np.float32)
    for b in range(B):
        out[b] = res[2 * b]["y"] + res[2 * b + 1]["y"]
    return out
